# revision 34
# baseline (speedup 1.0000x reference)
"""HSTU-style attention block (RoPE + multi-scale temporal agg + SDPA + LN + out-proj)
for Trainium2, data-parallel over batch across 8 NeuronCores.

v2 layout strategy (one batch element per core), all-bf16 dataflow:
  - host pre-transposes X and casts X/W/T/rope tables to bf16 (fp32 PSUM
    accumulation everywhere keeps the contraction error small)
  - Q/K/V projected into natural [s, h'] layout; RoPE applied as 4 DVE ops
    per chunk using a sign-folded sin table
  - temporal aggregation as banded matmuls against a host-built [S, S] matrix;
    (T@Q)^T / (T@K)^T land directly in SBUF head-pair tiles (no DRAM spill),
    V aggregated into [kpos, head, hd+1] tiles with a ones column so softmax
    denominators ride the PV matmul
  - attention: scores per head with K=64 contraction (lhsT/rhs are 64-row
    sub-slices of the packed pair tiles), two score chunks share a 2-bank PSUM
    tile so one Exp ACTIVATE covers [128, 1024] (the scalar engine is the
    attention bottleneck; it runs exps only)
  - softmax denominators: DVE reciprocal off the PSUM ones-row, then a PE
    outer-product broadcast into the free partitions of the PV tile; the
    normalize rides the attn_T eviction
  - Q/K temporal-agg matmuls are emitted inside the attention head loop so
    their PE work hides under the scalar-bound exp stream
  - LayerNorm stats accumulate on DVE during the head loop; partition sums via
    PE ones-matmuls; mean/rstd broadcast by PE outer products; normalize +
    out-projection pipelined per s-half
"""

import numpy as np
import ml_dtypes
import concourse.mybir as mybir
import concourse.tile as tile
from concourse import bacc
from concourse.bass_utils import run_bass_kernel_spmd

B, S, H, NH = 8, 1024, 1024, 16
HD = H // NH  # 64
P = 128
SO = S // P  # 8
HO = H // P  # 8
N_SCALES = 4
LN_EPS = 1e-5
F32 = mybir.dt.float32
F32R = mybir.dt.float32r
BF16 = mybir.dt.bfloat16
BF = ml_dtypes.bfloat16

N_CORES = 8
BAND = 12  # T[s', s] == 0 for |s' - s| > 11 (structural)


# ---------------------------------------------------------------- host helpers
def _softmax_np(x):
    x = np.asarray(x, np.float64)
    e = np.exp(x - x.max())
    return e / e.sum()


def _temporal_matrix(temporal_weights):
    """[S, S] matrix T with (T @ x) == temporal_agg(x) along the sequence axis."""
    w = _softmax_np(temporal_weights)
    T = np.eye(S, dtype=np.float64) * w[0]
    for scale in range(1, N_SCALES):
        p = max(1, S // (2 ** scale))
        k = S // p
        pool = np.zeros((p, S), dtype=np.float64)
        for j in range(p):
            pool[j, j * k:(j + 1) * k] = 1.0 / k
        coord = (np.arange(S, dtype=np.float64) + 0.5) * (p / S) - 0.5
        coord = np.clip(coord, 0.0, None)
        i0 = np.minimum(np.floor(coord).astype(np.int64), p - 1)
        i1 = np.minimum(i0 + 1, p - 1)
        lam = (coord - i0).astype(np.float32).astype(np.float64)
        interp = np.zeros((S, p), dtype=np.float64)
        interp[np.arange(S), i0] += 1.0 - lam
        interp[np.arange(S), i1] += lam
        T += w[scale] * (interp @ pool)
    return T.astype(np.float32)


def _rope_tables():
    inv_freq = 1.0 / (10000.0 ** (np.arange(0, HD, 2, dtype=np.float64) / HD))
    freqs = np.arange(S, dtype=np.float64)[:, None] * inv_freq[None, :]
    cos = np.repeat(np.cos(freqs), 2, axis=-1).astype(np.float32)  # [S, HD]
    sin = np.repeat(np.sin(freqs), 2, axis=-1).astype(np.float32)
    # fold the rotate-half sign into sin: rot = swap_halves(x) * sinp
    sinp = np.concatenate([-sin[:, :HD // 2], sin[:, HD // 2:]], axis=1)
    return cos, sinp


def _nat(x):
    """[S, D] -> [P, S//P, D] with x[so*P+p, d] = out[p, so, d]."""
    return np.ascontiguousarray(x.reshape(SO, P, x.shape[-1]).transpose(1, 0, 2))


def _xt_chunks(x):
    """[S, H] -> [P, SO, HO*P] with out[p, so, ho*P + i] = x[so*P + i, ho*P + p]."""
    return np.ascontiguousarray(
        x.reshape(SO, P, HO, P).transpose(3, 0, 2, 1).reshape(P, SO, H))


def _band_sos(o0, o1):
    """so chunks whose s-range intersects [o0-BAND, o1+BAND)."""
    return [so for so in range(SO)
            if so * P + P > o0 - BAND and so * P < o1 + BAND]


# ---------------------------------------------------------------- bass program
def _build_program():
    nc = bacc.Bacc("TRN2", target_bir_lowering=False, debug=False)

    d_xt = {a: nc.dram_tensor(f"xt_{a}", [P, SO, H], BF16, kind="ExternalInput")
            for a in ("v", "q", "k")}
    d_w = {a: nc.dram_tensor(f"w_{a}", [P, HO, H], BF16, kind="ExternalInput")
           for a in ("v", "q", "k", "o")}
    d_b = {a: nc.dram_tensor(f"b_{a}", [1, H], F32, kind="ExternalInput")
           for a in ("v", "q", "k", "o")}
    d_tt = nc.dram_tensor("tt", [P, SO, S], BF16, kind="ExternalInput")
    d_cos = nc.dram_tensor("cos_t", [P, SO, HD], BF16, kind="ExternalInput")
    d_sinp = nc.dram_tensor("sinp_t", [P, SO, HD], BF16, kind="ExternalInput")
    d_y = nc.dram_tensor("y", [P, SO, H], F32, kind="ExternalOutput")

    with tile.TileContext(nc) as tc:
        with (
            tc.tile_pool(name="const", bufs=1) as cpool,
            tc.tile_pool(name="wp", bufs=2) as wpool,
            tc.tile_pool(name="ap", bufs=2) as apool,
            tc.tile_pool(name="qk", bufs=2) as qkpool,
            tc.tile_pool(name="s4", bufs=2) as s4,
            tc.tile_pool(name="s2", bufs=2) as s2,
            tc.tile_pool(name="ep", bufs=4) as ep,
            tc.tile_pool(name="mm_ps", bufs=2, space="PSUM") as mmps,
            tc.tile_pool(name="sc_ps", bufs=2, space="PSUM") as scps,
            tc.tile_pool(name="pv_ps", bufs=2, space="PSUM") as pvps,
        ):
            # ---- persistent constants / state
            cos_t = cpool.tile([P, SO, HD], BF16, name="cos_t")
            sinp_t = cpool.tile([P, SO, HD], BF16, name="sinp_t")
            ones = cpool.tile([P, 1], F32, name="ones")
            nc.vector.memset(ones[:], 1.0)
            ones_col = cpool.tile([P, 1], F32R, name="ones_col")
            nc.vector.tensor_copy(ones_col[:], ones[:])
            eps_t = cpool.tile([P, 1], F32, name="eps_t")
            nc.vector.memset(eps_t[:], LN_EPS)

            v_ext = cpool.tile([P, SO, NH, HD + 1], BF16, name="v_ext")
            attn_T = cpool.tile([P, HO, S], BF16, name="attn_T")
            acc = cpool.tile([P, S], F32R, name="acc")
            acc2 = cpool.tile([P, S], F32R, name="acc2")

            def _rope_chunk(a_nat, so):
                ch = a_nat[:, so, :]
                ch3 = ch.rearrange("p (nh d) -> p nh d", d=HD)
                ch4 = ch.rearrange("p (nh hf dd) -> p nh hf dd", hf=2, dd=HD // 2)
                rot = s4.tile([P, H], BF16, tag="rot", bufs=2)
                rot4 = rot[:].rearrange("p (nh hf dd) -> p nh hf dd",
                                        hf=2, dd=HD // 2)
                sl = sinp_t[:, so, 0:HD // 2][:, None, :].to_broadcast(
                    (P, NH, HD // 2))
                sh = sinp_t[:, so, HD // 2:HD][:, None, :].to_broadcast(
                    (P, NH, HD // 2))
                cb = cos_t[:, so, :][:, None, :].to_broadcast((P, NH, HD))
                nc.vector.tensor_tensor(rot4[:, :, 0, :], ch4[:, :, 1, :], sl,
                                        mybir.AluOpType.mult)
                nc.vector.tensor_tensor(rot4[:, :, 1, :], ch4[:, :, 0, :], sh,
                                        mybir.AluOpType.mult)
                nc.vector.tensor_tensor(ch3[:], ch3[:], cb, mybir.AluOpType.mult)
                nc.vector.tensor_tensor(ch[:], ch[:], rot[:], mybir.AluOpType.add)

            def project(a, w_t, do_rope=False):
                """A_nat [P, SO, H] (bf16) = X @ W_a + b_a, optional fused RoPE."""
                brow = s2.tile([1, H], F32, tag="brow")
                nc.sync.dma_start(brow[:], d_b[a].ap())
                bb = s4.tile([P, H], F32, tag="bb", bufs=2)
                nc.gpsimd.partition_broadcast(bb[:], brow[:])
                a_nat = apool.tile([P, SO, H], BF16, tag="anat")
                for so in range(SO):
                    xt_c = s4.tile([P, HO, P], BF16, tag="xt", bufs=3)
                    nc.sync.dma_start(xt_c[:], d_xt[a].ap()[:, so, :])
                    for nh in range(2):
                        ps = mmps.tile([P, 512], F32, tag="mm")
                        for ko in range(HO):
                            nc.tensor.matmul(
                                ps[:], xt_c[:, ko, :],
                                w_t[:, ko, nh * 512:(nh + 1) * 512],
                                start=(ko == 0), stop=(ko == HO - 1))
                        nc.vector.tensor_tensor(
                            a_nat[:, so, nh * 512:(nh + 1) * 512], ps[:],
                            bb[:, nh * 512:(nh + 1) * 512], mybir.AluOpType.add)
                    if do_rope:
                        _rope_chunk(a_nat, so)
                return a_nat

            def t_agg_chunk(a_nat, tt, hc, tag, evict):
                """[P, S] tile = ((T @ A).T)[hc*P:(hc+1)*P, :] for one h-chunk."""
                pair = qkpool.tile([P, S], BF16, tag=tag, name=f"{tag}_{hc}")
                for sh2 in range(2):
                    sos = _band_sos(sh2 * 512, (sh2 + 1) * 512)
                    ps = mmps.tile([P, 512], F32, tag="mm")
                    for so in sos:
                        nc.tensor.matmul(
                            ps[:], a_nat[:, so, hc * P:(hc + 1) * P],
                            tt[:, so, sh2 * 512:(sh2 + 1) * 512],
                            start=(so == sos[0]), stop=(so == sos[-1]))
                    dst = pair[:, sh2 * 512:(sh2 + 1) * 512]
                    if evict == "vector":
                        nc.vector.tensor_copy(dst, ps[:])
                    elif evict == "gpsimd":
                        nc.gpsimd.tensor_copy(dst, ps[:])
                    else:
                        nc.scalar.copy(dst, ps[:])
                return pair

            def t_agg_v(v_nat, tt):
                """v_ext [P, SO, NH, HD+1] (bf16) = T @ V with ones column."""
                nc.vector.tensor_copy(
                    v_ext[:, :, :, HD:HD + 1],
                    ones[:, None, None, :].to_broadcast((P, SO, NH, 1)))
                for sc in range(SO):
                    sos = _band_sos(sc * P, (sc + 1) * P)
                    for dh in range(2):
                        ps = mmps.tile([P, 512], F32, tag="mm")
                        for so in sos:
                            nc.tensor.matmul(
                                ps[:], tt[:, so, sc * P:(sc + 1) * P],
                                v_nat[:, so, dh * 512:(dh + 1) * 512],
                                start=(so == sos[0]), stop=(so == sos[-1]))
                        pvw = ps[:].rearrange("p (nh d) -> p nh d", d=HD)
                        nc.scalar.copy(
                            v_ext[:, sc, dh * 8:(dh + 1) * 8, 0:HD], pvw)

            # ---- phase 1: V, Q, K  (projection + RoPE; V temporal agg)
            # weights + T matrix ride the scalar engine's HWDGE queue so the
            # activation chunks stream unobstructed on the sync queue
            w_v = wpool.tile([P, HO, H], BF16, tag="w")
            for ko in range(HO):
                nc.scalar.dma_start(w_v[:, ko, :], d_w["v"].ap()[:, ko, :])
            tt = cpool.tile([P, SO, S], BF16, name="tt_t")
            for so in range(SO):
                nc.scalar.dma_start(tt[:, so, :], d_tt.ap()[:, so, :])
            w_q = wpool.tile([P, HO, H], BF16, tag="w")
            nc.scalar.dma_start(w_q[:], d_w["q"].ap())
            v_nat = project("v", w_v)
            nc.sync.dma_start(cos_t[:], d_cos.ap())
            nc.sync.dma_start(sinp_t[:], d_sinp.ap())
            t_agg_v(v_nat, tt)

            q_nat = project("q", w_q, do_rope=True)
            w_k = wpool.tile([P, HO, H], BF16, tag="w")
            nc.scalar.dma_start(w_k[:], d_w["k"].ap())
            k_nat = project("k", w_k, do_rope=True)

            # prefetch out-projection weights + bias during attention
            wo_t = wpool.tile([P, HO, H], BF16, tag="w")
            nc.scalar.dma_start(wo_t[:], d_w["o"].ap())
            brow_o = s2.tile([1, H], F32, tag="brow")
            nc.sync.dma_start(brow_o[:], d_b["o"].ap())
            bo_b = s4.tile([P, H], F32, tag="bb", bufs=2)
            nc.gpsimd.partition_broadcast(bo_b[:], brow_o[:])

            # ---- phase 2: attention; Q/K temporal agg interleaved per chunk so
            # its PE work hides under the scalar-bound exp stream
            def attn_slot(h, qp, kp, q2):
                """Emit scores (one group ahead) + exp + PV for one (head,
                q-half); return a finalize closure (denominator broadcast +
                normalized eviction) to run one slot later so its PE matmul
                never stalls the in-order tensor stream."""
                hc, half = h // 2, h % 2
                koff = 64 * half
                kh = kp[koff:koff + 64, :]
                qh = qp[koff:koff + 64, q2 * 512:(q2 + 1) * 512]
                pv = pvps.tile([P, 512], F32, tag="pv", name=f"pv{h}_{q2}")
                ets = []
                for g in range(5):
                    if g < 4:
                        sc_ps = scps.tile([P, 2, 512], F32, tag="sc",
                                          name=f"sc{h}_{q2}_{g}")
                        for j in range(2):
                            kc = 2 * g + j
                            nc.tensor.matmul(
                                sc_ps[:, j, :], kh[:, kc * P:(kc + 1) * P],
                                qh, start=True, stop=True,
                                skip_group_check=True)
                        e_t = ep.tile([P, 2, 512], BF16, tag="et",
                                      name=f"et{h}_{q2}_{g}")
                        nc.scalar.activation(
                            e_t[:], sc_ps[:],
                            mybir.ActivationFunctionType.Exp, scale=0.125)
                        ets.append(e_t)
                    if g >= 1:
                        for j in range(2):
                            kc = 2 * (g - 1) + j
                            nc.tensor.matmul(
                                pv[0:HD + 1, :], v_ext[:, kc, h, :],
                                ets[g - 1][:, j, :], start=(kc == 0),
                                stop=(kc == SO - 1), skip_group_check=True)

                def finalize():
                    # softmax denominator: reciprocal of the PSUM ones-row,
                    # gpsimd partition-broadcast (gpsimd is otherwise idle),
                    # then one fused DVE multiply evicts + normalizes
                    qs = slice(q2 * 512, (q2 + 1) * 512)
                    dr = s2.tile([1, 512], F32, tag="dr", bufs=2,
                                 name=f"dr{h}_{q2}")
                    nc.vector.tensor_copy(dr[:], pv[HD:HD + 1, :])
                    drr = s2.tile([1, 512], F32, tag="drr", bufs=2,
                                  name=f"drr{h}_{q2}")
                    nc.vector.reciprocal_approx_fast(drr[:], dr[:])
                    rb = s4.tile([P, 512], F32, tag="rb", bufs=2,
                                 name=f"rb{h}_{q2}")
                    nc.gpsimd.partition_broadcast(rb[:], drr[:])
                    ev = attn_T[koff:koff + 64, hc, qs]
                    nc.vector.tensor_tensor(ev, pv[0:HD, :],
                                            rb[koff:koff + 64, :],
                                            mybir.AluOpType.mult)
                return finalize

            def ln_stats(hc):
                # LayerNorm statistics accumulate while attention continues
                if hc == 0:
                    nc.vector.tensor_copy(acc[:], attn_T[:, 0, :])
                    nc.vector.tensor_tensor(acc2[:], attn_T[:, 0, :],
                                            attn_T[:, 0, :],
                                            mybir.AluOpType.mult)
                else:
                    nc.vector.tensor_tensor(acc[:], acc[:], attn_T[:, hc, :],
                                            mybir.AluOpType.add)
                    sqc = s4.tile([P, S], F32R, tag="sqc", bufs=1,
                                  name=f"sqc{hc}")
                    nc.vector.tensor_tensor(sqc[:], attn_T[:, hc, :],
                                            attn_T[:, hc, :],
                                            mybir.AluOpType.mult)
                    nc.vector.tensor_tensor(acc2[:], acc2[:], sqc[:],
                                            mybir.AluOpType.add)

            pending = None
            for hc in range(HO):
                ev_eng = "vector" if hc > 0 else "scalar"
                qp = t_agg_chunk(q_nat, tt, hc, "qp", evict=ev_eng)
                kp = t_agg_chunk(k_nat, tt, hc, "kp", evict=ev_eng)
                for half in range(2):
                    for q2 in range(2):
                        fin = attn_slot(2 * hc + half, qp, kp, q2)
                        if pending is not None:
                            pending()
                        pending = fin
                if hc > 0:
                    ln_stats(hc - 1)
            pending()
            ln_stats(HO - 1)

            # ---- phase 3: LayerNorm (partition sums via PE ones-matmuls,
            # stats math on rows, gpsimd broadcasts) + out-projection,
            # pipelined per s-quarter; normalize split across DVE and gpsimd
            ln_out = apool.tile([P, HO, S], BF16, tag="anat")
            rowset = []
            for sh in range(2):
                ss = slice(sh * 512, (sh + 1) * 512)
                rows_ps = mmps.tile([P, 512], F32, tag="mm")
                nc.tensor.matmul(rows_ps[0:1, :], ones_col[:], acc[:, ss],
                                 start=True, stop=True, skip_group_check=True)
                rows_ps2 = mmps.tile([P, 512], F32, tag="mm")
                nc.tensor.matmul(rows_ps2[0:1, :], ones_col[:], acc2[:, ss],
                                 start=True, stop=True, skip_group_check=True)
                mu_r = s2.tile([1, 512], F32, tag="mu_r", bufs=1,
                               name=f"mu_r{sh}")
                ms_r = s2.tile([1, 512], F32, tag="ms_r", bufs=1,
                               name=f"ms_r{sh}")
                nc.vector.tensor_scalar_mul(mu_r[:], rows_ps[0:1, :], 1.0 / H)
                nc.vector.tensor_scalar_mul(ms_r[:], rows_ps2[0:1, :], 1.0 / H)
                m2 = s2.tile([1, 512], F32, tag="m2", bufs=1, name=f"m2_{sh}")
                nc.vector.tensor_tensor(m2[:], mu_r[:], mu_r[:],
                                        mybir.AluOpType.mult)
                nc.vector.tensor_tensor(ms_r[:], ms_r[:], m2[:],
                                        mybir.AluOpType.subtract)
                nc.scalar.activation(ms_r[:], ms_r[:],
                                     mybir.ActivationFunctionType.Sqrt,
                                     bias=eps_t[0:1, :])
                rstd_r = s2.tile([1, 512], F32, tag="rstd_r", bufs=2,
                                 name=f"rstd_r{sh}")
                nc.vector.reciprocal_approx_fast(rstd_r[:], ms_r[:])
                mrs_r = s2.tile([1, 512], F32, tag="mrs_r", bufs=2,
                                name=f"mrs_r{sh}")
                nc.vector.tensor_tensor(mrs_r[:], mu_r[:], rstd_r[:],
                                        mybir.AluOpType.mult)
                rowset.append((rstd_r, mrs_r))
            for qq in range(4):
                sh, qh2 = qq // 2, qq % 2
                rstd_r, mrs_r = rowset[sh]
                rsub = slice(qh2 * 256, qh2 * 256 + 256)
                ss = slice(qq * 256, (qq + 1) * 256)
                rstd_b = s4.tile([P, 256], F32, tag="lnb", bufs=4,
                                 name=f"rstd_b{qq}")
                nc.gpsimd.partition_broadcast(rstd_b[:], rstd_r[0:1, rsub])
                mrs_b = s4.tile([P, 256], F32, tag="lnb", bufs=4,
                                name=f"mrs_b{qq}")
                nc.gpsimd.partition_broadcast(mrs_b[:], mrs_r[0:1, rsub])
                for hc in range(HO):
                    # gamma/beta are folded into Wo/bo on the host
                    eng = nc.vector if hc < 6 else nc.gpsimd
                    t1 = s4.tile([P, 256], BF16, tag="t1", bufs=4,
                                 name=f"t1_{qq}_{hc}")
                    eng.tensor_tensor(t1[:], attn_T[:, hc, ss], rstd_b[:],
                                      mybir.AluOpType.mult)
                    eng.tensor_tensor(ln_out[:, hc, ss], t1[:], mrs_b[:],
                                      mybir.AluOpType.subtract)
                for so in range(qq * 2, qq * 2 + 2):
                    for nh in range(2):
                        ps = mmps.tile([P, 512], F32, tag="mm")
                        for hc in range(HO):
                            nc.tensor.matmul(
                                ps[:], ln_out[:, hc, so * P:(so + 1) * P],
                                wo_t[:, hc, nh * 512:(nh + 1) * 512],
                                start=(hc == 0), stop=(hc == HO - 1))
                        ych = s2.tile([P, 512], F32, tag="ych", bufs=3)
                        nc.vector.tensor_tensor(ych[:], ps[:],
                                                bo_b[:, nh * 512:(nh + 1) * 512],
                                                mybir.AluOpType.add)
                        dq = nc.sync if (so + nh) % 2 == 0 else nc.scalar
                        dq.dma_start(
                            d_y.ap()[:, so, nh * 512:(nh + 1) * 512], ych[:])

    nc.compile()
    return nc


_NC = None


def _get_nc():
    global _NC
    if _NC is None:
        _NC = _build_program()
    return _NC


def _host_inputs(query, key, value, Wq, bq, Wk, bk, Wv, bv, Wo, bo,
                 temporal_weights, ln_gamma, ln_beta):
    T = _temporal_matrix(temporal_weights)
    tt_host = np.ascontiguousarray(  # TT[p, so, s'] = T[s', so*P+p]
        T.T.reshape(SO, P, S).transpose(1, 0, 2)).astype(BF)
    cos, sinp = _rope_tables()
    # fold LayerNorm gamma/beta into the out-projection
    g = np.asarray(ln_gamma, np.float64)
    Wo64 = np.asarray(Wo, np.float64)
    Wo_f = (g[:, None] * Wo64).astype(np.float32)
    bo_f = (np.asarray(ln_beta, np.float64) @ Wo64
            + np.asarray(bo, np.float64)).astype(np.float32)
    common = {
        "w_v": _nat(np.asarray(Wv, np.float32)).astype(BF),
        "w_q": _nat(np.asarray(Wq, np.float32)).astype(BF),
        "w_k": _nat(np.asarray(Wk, np.float32)).astype(BF),
        "w_o": _nat(Wo_f).astype(BF),
        "b_v": np.asarray(bv, np.float32).reshape(1, H),
        "b_q": np.asarray(bq, np.float32).reshape(1, H),
        "b_k": np.asarray(bk, np.float32).reshape(1, H),
        "b_o": bo_f.reshape(1, H),
        "tt": tt_host,
        "cos_t": _nat(cos).astype(BF),
        "sinp_t": _nat(sinp).astype(BF),
    }
    in_maps = []
    for c in range(N_CORES):
        m = dict(common)
        m["xt_q"] = _xt_chunks(np.asarray(query[c], np.float32)).astype(BF)
        m["xt_k"] = _xt_chunks(np.asarray(key[c], np.float32)).astype(BF)
        m["xt_v"] = _xt_chunks(np.asarray(value[c], np.float32)).astype(BF)
        in_maps.append(m)
    return in_maps


def kernel(query, key, value, Wq, bq, Wk, bk, Wv, bv, Wo, bo,
           temporal_weights, ln_gamma, ln_beta):
    in_maps = _host_inputs(query, key, value, Wq, bq, Wk, bk, Wv, bv, Wo, bo,
                           temporal_weights, ln_gamma, ln_beta)
    nc = _get_nc()
    res = run_bass_kernel_spmd(nc, in_maps, list(range(N_CORES)))
    out = np.empty((B, S, H), np.float32)
    for c in range(N_CORES):
        y = res.results[c]["y"]  # [P, SO, H]
        out[c] = y.transpose(1, 0, 2).reshape(S, H)
    return out


# revision 38
# speedup vs baseline: 1.0634x; 1.0634x over previous
"""HSTU-style attention block (RoPE + multi-scale temporal agg + SDPA + LN + out-proj)
for Trainium2, data-parallel over batch across 8 NeuronCores.

v2 layout strategy (one batch element per core), all-bf16 dataflow:
  - host pre-transposes X and casts X/W/T/rope tables to bf16 (fp32 PSUM
    accumulation everywhere keeps the contraction error small)
  - Q/K/V projected into natural [s, h'] layout; RoPE applied as 4 DVE ops
    per chunk using a sign-folded sin table
  - temporal aggregation as banded matmuls against a host-built [S, S] matrix;
    (T@Q)^T / (T@K)^T land directly in SBUF head-pair tiles (no DRAM spill),
    V aggregated into [kpos, head, hd+1] tiles with a ones column so softmax
    denominators ride the PV matmul
  - attention: scores per head with K=64 contraction (lhsT/rhs are 64-row
    sub-slices of the packed pair tiles), two score chunks share a 2-bank PSUM
    tile so one Exp ACTIVATE covers [128, 1024] (the scalar engine is the
    attention bottleneck; it runs exps only)
  - softmax denominators: DVE reciprocal off the PSUM ones-row, then a PE
    outer-product broadcast into the free partitions of the PV tile; the
    normalize rides the attn_T eviction
  - Q/K temporal-agg matmuls are emitted inside the attention head loop so
    their PE work hides under the scalar-bound exp stream
  - LayerNorm stats accumulate on DVE during the head loop; partition sums via
    PE ones-matmuls; mean/rstd broadcast by PE outer products; normalize +
    out-projection pipelined per s-half
"""

import numpy as np
import ml_dtypes
import concourse.mybir as mybir
import concourse.tile as tile
from concourse import bacc
from concourse.bass_utils import run_bass_kernel_spmd

B, S, H, NH = 8, 1024, 1024, 16
HD = H // NH  # 64
P = 128
SO = S // P  # 8
HO = H // P  # 8
N_SCALES = 4
LN_EPS = 1e-5
F32 = mybir.dt.float32
F32R = mybir.dt.float32r
BF16 = mybir.dt.bfloat16
BF = ml_dtypes.bfloat16

N_CORES = 8
BAND = 12  # T[s', s] == 0 for |s' - s| > 11 (structural)


# ---------------------------------------------------------------- host helpers
def _softmax_np(x):
    x = np.asarray(x, np.float64)
    e = np.exp(x - x.max())
    return e / e.sum()


def _temporal_matrix(temporal_weights):
    """[S, S] matrix T with (T @ x) == temporal_agg(x) along the sequence axis."""
    w = _softmax_np(temporal_weights)
    T = np.eye(S, dtype=np.float64) * w[0]
    for scale in range(1, N_SCALES):
        p = max(1, S // (2 ** scale))
        k = S // p
        pool = np.zeros((p, S), dtype=np.float64)
        for j in range(p):
            pool[j, j * k:(j + 1) * k] = 1.0 / k
        coord = (np.arange(S, dtype=np.float64) + 0.5) * (p / S) - 0.5
        coord = np.clip(coord, 0.0, None)
        i0 = np.minimum(np.floor(coord).astype(np.int64), p - 1)
        i1 = np.minimum(i0 + 1, p - 1)
        lam = (coord - i0).astype(np.float32).astype(np.float64)
        interp = np.zeros((S, p), dtype=np.float64)
        interp[np.arange(S), i0] += 1.0 - lam
        interp[np.arange(S), i1] += lam
        T += w[scale] * (interp @ pool)
    return T.astype(np.float32)


def _rope_tables():
    inv_freq = 1.0 / (10000.0 ** (np.arange(0, HD, 2, dtype=np.float64) / HD))
    freqs = np.arange(S, dtype=np.float64)[:, None] * inv_freq[None, :]
    cos = np.repeat(np.cos(freqs), 2, axis=-1).astype(np.float32)  # [S, HD]
    sin = np.repeat(np.sin(freqs), 2, axis=-1).astype(np.float32)
    # fold the rotate-half sign into sin: rot = swap_halves(x) * sinp
    sinp = np.concatenate([-sin[:, :HD // 2], sin[:, HD // 2:]], axis=1)
    return cos, sinp


def _nat(x):
    """[S, D] -> [P, S//P, D] with x[so*P+p, d] = out[p, so, d]."""
    return np.ascontiguousarray(x.reshape(SO, P, x.shape[-1]).transpose(1, 0, 2))


def _xt_chunks(x):
    """[S, H] -> [P, SO, HO*P] with out[p, so, ho*P + i] = x[so*P + i, ho*P + p]."""
    return np.ascontiguousarray(
        x.reshape(SO, P, HO, P).transpose(3, 0, 2, 1).reshape(P, SO, H))


def _band_sos(o0, o1):
    """so chunks whose s-range intersects [o0-BAND, o1+BAND)."""
    return [so for so in range(SO)
            if so * P + P > o0 - BAND and so * P < o1 + BAND]


# ---------------------------------------------------------------- bass program
def _build_program():
    nc = bacc.Bacc("TRN2", target_bir_lowering=False, debug=False)

    d_xt = {a: nc.dram_tensor(f"xt_{a}", [P, SO, H], BF16, kind="ExternalInput")
            for a in ("v", "q", "k")}
    d_w = {a: nc.dram_tensor(f"w_{a}", [P, HO, H], BF16, kind="ExternalInput")
           for a in ("v", "q", "k", "o")}
    d_b = {a: nc.dram_tensor(f"b_{a}", [1, H], F32, kind="ExternalInput")
           for a in ("v", "q", "k", "o")}
    d_tt = nc.dram_tensor("tt", [P, SO, S], BF16, kind="ExternalInput")
    d_cos = nc.dram_tensor("cos_t", [P, SO, HD], BF16, kind="ExternalInput")
    d_sinp = nc.dram_tensor("sinp_t", [P, SO, HD], BF16, kind="ExternalInput")
    d_y = nc.dram_tensor("y", [P, SO, H], F32, kind="ExternalOutput")

    with tile.TileContext(nc) as tc:
        with (
            tc.tile_pool(name="const", bufs=1) as cpool,
            tc.tile_pool(name="wp", bufs=2) as wpool,
            tc.tile_pool(name="ap", bufs=2) as apool,
            tc.tile_pool(name="qk", bufs=2) as qkpool,
            tc.tile_pool(name="s4", bufs=2) as s4,
            tc.tile_pool(name="s2", bufs=2) as s2,
            tc.tile_pool(name="ep", bufs=4) as ep,
            tc.tile_pool(name="mm_ps", bufs=2, space="PSUM") as mmps,
            tc.tile_pool(name="sc_ps", bufs=2, space="PSUM") as scps,
            tc.tile_pool(name="pv_ps", bufs=2, space="PSUM") as pvps,
        ):
            # ---- persistent constants / state
            cos_t = cpool.tile([P, SO, HD], BF16, name="cos_t")
            sinp_t = cpool.tile([P, SO, HD], BF16, name="sinp_t")
            ones = cpool.tile([P, 1], F32, name="ones")
            nc.vector.memset(ones[:], 1.0)
            ones_col = cpool.tile([P, 1], F32R, name="ones_col")
            nc.vector.tensor_copy(ones_col[:], ones[:])
            eps_t = cpool.tile([P, 1], F32, name="eps_t")
            nc.vector.memset(eps_t[:], LN_EPS)

            v_ext = cpool.tile([P, SO, NH, HD + 1], BF16, name="v_ext")
            attn_T = cpool.tile([P, HO, S], BF16, name="attn_T")
            acc = cpool.tile([P, S], F32R, name="acc")
            acc2 = cpool.tile([P, S], F32R, name="acc2")

            def _rope_chunk(a_nat, so):
                ch = a_nat[:, so, :]
                ch3 = ch.rearrange("p (nh d) -> p nh d", d=HD)
                ch4 = ch.rearrange("p (nh hf dd) -> p nh hf dd", hf=2, dd=HD // 2)
                rot = s4.tile([P, H], BF16, tag="rot", bufs=2)
                rot4 = rot[:].rearrange("p (nh hf dd) -> p nh hf dd",
                                        hf=2, dd=HD // 2)
                sl = sinp_t[:, so, 0:HD // 2][:, None, :].to_broadcast(
                    (P, NH, HD // 2))
                sh = sinp_t[:, so, HD // 2:HD][:, None, :].to_broadcast(
                    (P, NH, HD // 2))
                cb = cos_t[:, so, :][:, None, :].to_broadcast((P, NH, HD))
                nc.vector.tensor_tensor(rot4[:, :, 0, :], ch4[:, :, 1, :], sl,
                                        mybir.AluOpType.mult)
                nc.vector.tensor_tensor(rot4[:, :, 1, :], ch4[:, :, 0, :], sh,
                                        mybir.AluOpType.mult)
                nc.vector.tensor_tensor(ch3[:], ch3[:], cb, mybir.AluOpType.mult)
                nc.vector.tensor_tensor(ch[:], ch[:], rot[:], mybir.AluOpType.add)

            def project(a, w_t, do_rope=False, xt_eng=None):
                """A_nat [P, SO, H] (bf16) = X @ W_a + b_a, optional fused RoPE."""
                xt_eng = xt_eng or nc.sync
                brow = s2.tile([1, H], F32, tag="brow")
                nc.sync.dma_start(brow[:], d_b[a].ap())
                bb = s4.tile([P, H], F32, tag="bb", bufs=2)
                nc.gpsimd.partition_broadcast(bb[:], brow[:])
                a_nat = apool.tile([P, SO, H], BF16, tag="anat")
                for so in range(SO):
                    xt_c = s4.tile([P, HO, P], BF16, tag="xt", bufs=3)
                    xt_eng.dma_start(xt_c[:], d_xt[a].ap()[:, so, :])
                    for nh in range(2):
                        ps = mmps.tile([P, 512], F32, tag="mm")
                        for ko in range(HO):
                            nc.tensor.matmul(
                                ps[:], xt_c[:, ko, :],
                                w_t[:, ko, nh * 512:(nh + 1) * 512],
                                start=(ko == 0), stop=(ko == HO - 1))
                        nc.vector.tensor_tensor(
                            a_nat[:, so, nh * 512:(nh + 1) * 512], ps[:],
                            bb[:, nh * 512:(nh + 1) * 512], mybir.AluOpType.add)
                    if do_rope:
                        _rope_chunk(a_nat, so)
                return a_nat

            def t_agg_chunk(a_nat, tt, hc, tag, evict):
                """[P, S] tile = ((T @ A).T)[hc*P:(hc+1)*P, :] for one h-chunk."""
                pair = qkpool.tile([P, S], BF16, tag=tag, name=f"{tag}_{hc}")
                for sh2 in range(2):
                    sos = _band_sos(sh2 * 512, (sh2 + 1) * 512)
                    ps = mmps.tile([P, 512], F32, tag="mm")
                    for so in sos:
                        nc.tensor.matmul(
                            ps[:], a_nat[:, so, hc * P:(hc + 1) * P],
                            tt[:, so, sh2 * 512:(sh2 + 1) * 512],
                            start=(so == sos[0]), stop=(so == sos[-1]))
                    dst = pair[:, sh2 * 512:(sh2 + 1) * 512]
                    if evict == "vector":
                        nc.vector.tensor_copy(dst, ps[:])
                    elif evict == "gpsimd":
                        nc.gpsimd.tensor_copy(dst, ps[:])
                    else:
                        nc.scalar.copy(dst, ps[:])
                return pair

            def t_agg_v(v_nat, tt):
                """v_ext [P, SO, NH, HD+1] (bf16) = T @ V with ones column."""
                nc.vector.tensor_copy(
                    v_ext[:, :, :, HD:HD + 1],
                    ones[:, None, None, :].to_broadcast((P, SO, NH, 1)))
                for sc in range(SO):
                    sos = _band_sos(sc * P, (sc + 1) * P)
                    for dh in range(2):
                        ps = mmps.tile([P, 512], F32, tag="mm")
                        for so in sos:
                            nc.tensor.matmul(
                                ps[:], tt[:, so, sc * P:(sc + 1) * P],
                                v_nat[:, so, dh * 512:(dh + 1) * 512],
                                start=(so == sos[0]), stop=(so == sos[-1]))
                        pvw = ps[:].rearrange("p (nh d) -> p nh d", d=HD)
                        nc.scalar.copy(
                            v_ext[:, sc, dh * 8:(dh + 1) * 8, 0:HD], pvw)

            # ---- phase 1: V, Q, K  (projection + RoPE; V temporal agg)
            # weights + T matrix ride the scalar engine's HWDGE queue so the
            # activation chunks stream unobstructed on the sync queue
            w_v = wpool.tile([P, HO, H], BF16, tag="w")
            for ko in range(HO):
                nc.scalar.dma_start(w_v[:, ko, :], d_w["v"].ap()[:, ko, :])
            w_q = wpool.tile([P, HO, H], BF16, tag="w")
            nc.scalar.dma_start(w_q[:], d_w["q"].ap())
            v_nat = project("v", w_v)
            tt = cpool.tile([P, SO, S], BF16, name="tt_t")
            for so in range(SO):
                nc.sync.dma_start(tt[:, so, :], d_tt.ap()[:, so, :])
            nc.sync.dma_start(cos_t[:], d_cos.ap())
            nc.sync.dma_start(sinp_t[:], d_sinp.ap())
            t_agg_v(v_nat, tt)

            q_nat = project("q", w_q, do_rope=True, xt_eng=nc.scalar)
            w_k = wpool.tile([P, HO, H], BF16, tag="w")
            nc.scalar.dma_start(w_k[:], d_w["k"].ap())
            k_nat = project("k", w_k, do_rope=True, xt_eng=nc.scalar)

            # prefetch out-projection weights + bias during attention
            wo_t = wpool.tile([P, HO, H], BF16, tag="w")
            nc.scalar.dma_start(wo_t[:], d_w["o"].ap())
            brow_o = s2.tile([1, H], F32, tag="brow")
            nc.sync.dma_start(brow_o[:], d_b["o"].ap())
            bo_b = s4.tile([P, H], F32, tag="bb", bufs=2)
            nc.gpsimd.partition_broadcast(bo_b[:], brow_o[:])

            # ---- phase 2: attention; Q/K temporal agg interleaved per chunk so
            # its PE work hides under the scalar-bound exp stream
            def attn_slot(h, qp, kp, q2):
                """Emit scores (one group ahead) + exp + PV for one (head,
                q-half); return a finalize closure (denominator broadcast +
                normalized eviction) to run one slot later so its PE matmul
                never stalls the in-order tensor stream."""
                hc, half = h // 2, h % 2
                koff = 64 * half
                kh = kp[koff:koff + 64, :]
                qh = qp[koff:koff + 64, q2 * 512:(q2 + 1) * 512]
                pv = pvps.tile([P, 512], F32, tag="pv", name=f"pv{h}_{q2}")
                ets = []
                for g in range(5):
                    if g < 4:
                        sc_ps = scps.tile([P, 2, 512], F32, tag="sc",
                                          name=f"sc{h}_{q2}_{g}")
                        for j in range(2):
                            kc = 2 * g + j
                            nc.tensor.matmul(
                                sc_ps[:, j, :], kh[:, kc * P:(kc + 1) * P],
                                qh, start=True, stop=True,
                                skip_group_check=True)
                        e_t = ep.tile([P, 2, 512], BF16, tag="et",
                                      name=f"et{h}_{q2}_{g}")
                        nc.scalar.activation(
                            e_t[:], sc_ps[:],
                            mybir.ActivationFunctionType.Exp, scale=0.125)
                        ets.append(e_t)
                    if g >= 1:
                        for j in range(2):
                            kc = 2 * (g - 1) + j
                            nc.tensor.matmul(
                                pv[0:HD + 1, :], v_ext[:, kc, h, :],
                                ets[g - 1][:, j, :], start=(kc == 0),
                                stop=(kc == SO - 1), skip_group_check=True)

                def finalize():
                    # softmax denominator: reciprocal of the PSUM ones-row,
                    # gpsimd partition-broadcast (gpsimd is otherwise idle),
                    # then one fused DVE multiply evicts + normalizes
                    qs = slice(q2 * 512, (q2 + 1) * 512)
                    dr = s2.tile([1, 512], F32, tag="dr", bufs=2,
                                 name=f"dr{h}_{q2}")
                    nc.vector.tensor_copy(dr[:], pv[HD:HD + 1, :])
                    drr = s2.tile([1, 512], F32, tag="drr", bufs=2,
                                  name=f"drr{h}_{q2}")
                    nc.vector.reciprocal_approx_fast(drr[:], dr[:])
                    rb = s4.tile([P, 512], F32, tag="rb", bufs=2,
                                 name=f"rb{h}_{q2}")
                    nc.gpsimd.partition_broadcast(rb[:], drr[:])
                    ev = attn_T[koff:koff + 64, hc, qs]
                    nc.vector.tensor_tensor(ev, pv[0:HD, :],
                                            rb[koff:koff + 64, :],
                                            mybir.AluOpType.mult)
                return finalize

            def ln_stats(hc):
                # LayerNorm statistics accumulate while attention continues
                if hc == 0:
                    nc.vector.tensor_copy(acc[:], attn_T[:, 0, :])
                    nc.vector.tensor_tensor(acc2[:], attn_T[:, 0, :],
                                            attn_T[:, 0, :],
                                            mybir.AluOpType.mult)
                else:
                    nc.vector.tensor_tensor(acc[:], acc[:], attn_T[:, hc, :],
                                            mybir.AluOpType.add)
                    sqc = s4.tile([P, S], F32R, tag="sqc", bufs=1,
                                  name=f"sqc{hc}")
                    nc.vector.tensor_tensor(sqc[:], attn_T[:, hc, :],
                                            attn_T[:, hc, :],
                                            mybir.AluOpType.mult)
                    nc.vector.tensor_tensor(acc2[:], acc2[:], sqc[:],
                                            mybir.AluOpType.add)

            pending = None
            for hc in range(HO):
                ev_eng = "vector" if hc > 0 else "scalar"
                qp = t_agg_chunk(q_nat, tt, hc, "qp", evict=ev_eng)
                kp = t_agg_chunk(k_nat, tt, hc, "kp", evict=ev_eng)
                for half in range(2):
                    for q2 in range(2):
                        fin = attn_slot(2 * hc + half, qp, kp, q2)
                        if pending is not None:
                            pending()
                        pending = fin
                if hc > 0:
                    ln_stats(hc - 1)
            pending()
            ln_stats(HO - 1)

            # keep the PE activity monitor warm through the serial LN-rows
            # chain (an idle window here halves the clock for the whole tail)
            def pe_warm(n, tag):
                dmy = mmps.tile([P, 512], F32, tag="mm", name=f"warm_{tag}")
                for i in range(n):
                    nc.tensor.matmul(dmy[0:1, :], ones_col[:], acc[:, 0:512],
                                     start=True, stop=True,
                                     skip_group_check=True)

            # ---- phase 3: LayerNorm (partition sums via PE ones-matmuls,
            # stats math on rows, gpsimd broadcasts) + out-projection,
            # pipelined per s-quarter
            pe_warm(30, "stats")
            ln_out = apool.tile([P, HO, S], BF16, tag="anat")
            rowset = []
            for sh in range(2):
                ss = slice(sh * 512, (sh + 1) * 512)
                rows_ps = mmps.tile([P, 512], F32, tag="mm")
                nc.tensor.matmul(rows_ps[0:1, :], ones_col[:], acc[:, ss],
                                 start=True, stop=True, skip_group_check=True)
                rows_ps2 = mmps.tile([P, 512], F32, tag="mm")
                nc.tensor.matmul(rows_ps2[0:1, :], ones_col[:], acc2[:, ss],
                                 start=True, stop=True, skip_group_check=True)
                mu_r = s2.tile([1, 512], F32, tag="mu_r", bufs=1,
                               name=f"mu_r{sh}")
                ms_r = s2.tile([1, 512], F32, tag="ms_r", bufs=1,
                               name=f"ms_r{sh}")
                nc.vector.tensor_scalar_mul(mu_r[:], rows_ps[0:1, :], 1.0 / H)
                nc.vector.tensor_scalar_mul(ms_r[:], rows_ps2[0:1, :], 1.0 / H)
                m2 = s2.tile([1, 512], F32, tag="m2", bufs=1, name=f"m2_{sh}")
                nc.vector.tensor_tensor(m2[:], mu_r[:], mu_r[:],
                                        mybir.AluOpType.mult)
                nc.vector.tensor_tensor(ms_r[:], ms_r[:], m2[:],
                                        mybir.AluOpType.subtract)
                nc.scalar.activation(ms_r[:], ms_r[:],
                                     mybir.ActivationFunctionType.Sqrt,
                                     bias=eps_t[0:1, :])
                rstd_r = s2.tile([1, 512], F32, tag="rstd_r", bufs=2,
                                 name=f"rstd_r{sh}")
                nc.vector.reciprocal_approx_fast(rstd_r[:], ms_r[:])
                mrs_r = s2.tile([1, 512], F32, tag="mrs_r", bufs=2,
                                name=f"mrs_r{sh}")
                nc.vector.tensor_tensor(mrs_r[:], mu_r[:], rstd_r[:],
                                        mybir.AluOpType.mult)
                rowset.append((rstd_r, mrs_r))
                pe_warm(12, f"rows{sh}")
            for qq in range(4):
                sh, qh2 = qq // 2, qq % 2
                rstd_r, mrs_r = rowset[sh]
                rsub = slice(qh2 * 256, qh2 * 256 + 256)
                ss = slice(qq * 256, (qq + 1) * 256)
                rstd_b = s4.tile([P, 256], F32, tag="lnb", bufs=4,
                                 name=f"rstd_b{qq}")
                nc.gpsimd.partition_broadcast(rstd_b[:], rstd_r[0:1, rsub])
                mrs_b = s4.tile([P, 256], F32, tag="lnb", bufs=4,
                                name=f"mrs_b{qq}")
                nc.gpsimd.partition_broadcast(mrs_b[:], mrs_r[0:1, rsub])
                for hc in range(HO):
                    # gamma/beta are folded into Wo/bo on the host
                    t1 = s4.tile([P, 256], BF16, tag="t1", bufs=4,
                                 name=f"t1_{qq}_{hc}")
                    nc.vector.tensor_tensor(t1[:], attn_T[:, hc, ss],
                                            rstd_b[:], mybir.AluOpType.mult)
                    nc.vector.tensor_tensor(ln_out[:, hc, ss], t1[:], mrs_b[:],
                                            mybir.AluOpType.subtract)
                if qq == 0:
                    pe_warm(16, "norm0")
                for so in range(qq * 2, qq * 2 + 2):
                    for nh in range(2):
                        ps = mmps.tile([P, 512], F32, tag="mm")
                        for hc in range(HO):
                            nc.tensor.matmul(
                                ps[:], ln_out[:, hc, so * P:(so + 1) * P],
                                wo_t[:, hc, nh * 512:(nh + 1) * 512],
                                start=(hc == 0), stop=(hc == HO - 1))
                        ych = s2.tile([P, 512], F32, tag="ych", bufs=3)
                        nc.vector.tensor_tensor(ych[:], ps[:],
                                                bo_b[:, nh * 512:(nh + 1) * 512],
                                                mybir.AluOpType.add)
                        dq = nc.sync if (so + nh) % 2 == 0 else nc.scalar
                        dq.dma_start(
                            d_y.ap()[:, so, nh * 512:(nh + 1) * 512], ych[:])

    nc.compile()
    return nc


_NC = None


def _get_nc():
    global _NC
    if _NC is None:
        _NC = _build_program()
    return _NC


def _host_inputs(query, key, value, Wq, bq, Wk, bk, Wv, bv, Wo, bo,
                 temporal_weights, ln_gamma, ln_beta):
    T = _temporal_matrix(temporal_weights)
    tt_host = np.ascontiguousarray(  # TT[p, so, s'] = T[s', so*P+p]
        T.T.reshape(SO, P, S).transpose(1, 0, 2)).astype(BF)
    cos, sinp = _rope_tables()
    # fold LayerNorm gamma/beta into the out-projection
    g = np.asarray(ln_gamma, np.float64)
    Wo64 = np.asarray(Wo, np.float64)
    Wo_f = (g[:, None] * Wo64).astype(np.float32)
    bo_f = (np.asarray(ln_beta, np.float64) @ Wo64
            + np.asarray(bo, np.float64)).astype(np.float32)
    common = {
        "w_v": _nat(np.asarray(Wv, np.float32)).astype(BF),
        "w_q": _nat(np.asarray(Wq, np.float32)).astype(BF),
        "w_k": _nat(np.asarray(Wk, np.float32)).astype(BF),
        "w_o": _nat(Wo_f).astype(BF),
        "b_v": np.asarray(bv, np.float32).reshape(1, H),
        "b_q": np.asarray(bq, np.float32).reshape(1, H),
        "b_k": np.asarray(bk, np.float32).reshape(1, H),
        "b_o": bo_f.reshape(1, H),
        "tt": tt_host,
        "cos_t": _nat(cos).astype(BF),
        "sinp_t": _nat(sinp).astype(BF),
    }
    in_maps = []
    for c in range(N_CORES):
        m = dict(common)
        m["xt_q"] = _xt_chunks(np.asarray(query[c], np.float32)).astype(BF)
        m["xt_k"] = _xt_chunks(np.asarray(key[c], np.float32)).astype(BF)
        m["xt_v"] = _xt_chunks(np.asarray(value[c], np.float32)).astype(BF)
        in_maps.append(m)
    return in_maps


def kernel(query, key, value, Wq, bq, Wk, bk, Wv, bv, Wo, bo,
           temporal_weights, ln_gamma, ln_beta):
    in_maps = _host_inputs(query, key, value, Wq, bq, Wk, bk, Wv, bv, Wo, bo,
                           temporal_weights, ln_gamma, ln_beta)
    nc = _get_nc()
    res = run_bass_kernel_spmd(nc, in_maps, list(range(N_CORES)))
    out = np.empty((B, S, H), np.float32)
    for c in range(N_CORES):
        y = res.results[c]["y"]  # [P, SO, H]
        out[c] = y.transpose(1, 0, 2).reshape(S, H)
    return out
